# revision 10
# baseline (speedup 1.0000x reference)
"""BCMP layer (GNN message passing) on 8 Trainium2 NeuronCores.

Math (see harness reference):
    out = (ahat(x@WX) + bhat(bcf@WZ) + ahat(bhat(bcf@Walpha))) / 3
By linearity of ahat:  out = ahat(G)/3 + bhat(bcf@WZ)/3  with
    G = x@WX + bhat(bcf@Walpha)
    ahat(G) = d*segsum_dest(d[col]*G[col]) + d^2*G ,  d = deg^-0.5

Two SPMD launches over 8 cores (destination nodes sharded, 12500/core):
  Launch 1 (node phase): per-core slice of Gs = d*G (bf16) and
  R = (d^2*G + Zprime)/3 (f32), in natural node order.
  Host: concat Gs slices, permute R into window-slot order.
  Launch 2 (edge phase): each core owns ~E/8 edges grouped into
  NW=100 windows of 128 destination slots.  Nodes are packed into
  windows by a 4-dimensional capacity-constrained LPT so that every
  (window, source-bank) cell holds at most CB*128 edges, with CB=4
  fixed -- this keeps the program uniform across cores and pads the
  gather stream by only ~2-3%% (vs ~25%% for a global max ceiling).
  Messages Gs[col] are fetched with dma_gather (int16 indices, 4
  source banks of 25000 rows) in 8 large calls per bank; one-hot
  segment-sum matmuls accumulate each window in PSUM; a fused
  scalar_tensor_tensor computes out = (d/3)*agg + R.

All floating point math runs on device; the host only does integer
index manipulation (bincount/argsort/packing) and data movement.
"""

import numpy as np
import ml_dtypes

import concourse.bacc as bacc
import concourse.mybir as mybir
from concourse.tile import TileContext
from concourse.bass_utils import run_bass_kernel_spmd

N = 100000
E = 1600000
M = 1000
D = 128
NCORES = 8
NC = N // NCORES            # 12500 nodes per core
P = 128

# ---- launch 1 (node phase) geometry: natural order, 98 windows ----
NW1 = NC // P + (1 if NC % P else 0)   # 98
SLOTS1 = NW1 * P                       # 12544
MPAD = 1024
GRP1 = 14                              # windows per bc-gather call
NGRP1 = NW1 // GRP1                    # 7

# ---- launch 2 (edge phase) geometry ----
NB = 4                      # source banks (int16 index reach)
BANK = 25000                # rows per bank
NW = 100                    # windows per core
SLOTS = NW * P              # 12800 slots per core
CB = 4                      # preferred chunks of 128 edges per (window, bank)
CB_FALLBACK = 5
NI2 = 6400                  # idxs per gather call


def _geom(cb):
    cap = cb * P            # edge capacity per (window, bank)
    nch = NW * cb           # chunks per bank stream
    stream = nch * P        # gather positions per bank
    calls_pb = stream // NI2
    return cap, nch, stream, calls_pb

INV3 = 1.0 / 3.0
C1 = 2.0 ** -0.5

F32 = mybir.dt.float32
BF16 = mybir.dt.bfloat16
I16 = mybir.dt.int16
AOP = mybir.AluOpType
ACT = mybir.ActivationFunctionType
BF16NP = ml_dtypes.bfloat16

CORE_IDS = list(range(NCORES))

LAST_RESULTS = []           # test harness hook

_kernel_cache = {}


def _wrap16(vals, n):
    """Pack flat idx list (len n) into dma_gather's [128, n//16] int16 layout:
    flat i -> [i % 16, i // 16], replicated across the 8 groups of 16
    partitions."""
    lay = np.zeros((16, n // 16), np.int16)
    lay[np.arange(n) % 16, np.arange(n) // 16] = vals
    return np.tile(lay, (8, 1))


def _build_launch1():
    nc = bacc.Bacc()
    xT = nc.declare_dram_parameter("xT", [P, SLOTS1], F32, isOutput=False)
    WXp = nc.declare_dram_parameter("WX", [P, D], F32, isOutput=False)
    WAp = nc.declare_dram_parameter("WA", [P, D], F32, isOutput=False)
    WZp = nc.declare_dram_parameter("WZ", [P, D], F32, isOutput=False)
    bcfT = nc.declare_dram_parameter("bcfT", [P, MPAD], F32, isOutput=False)
    NI1 = GRP1 * P
    aidx = nc.declare_dram_parameter("aidx16", [P, NGRP1 * (NI1 // 16)], I16,
                                     isOutput=False)
    degp = nc.declare_dram_parameter("deg", [P, NW1], F32, isOutput=False)
    dcntp = nc.declare_dram_parameter("dcnt", [P, 8], F32, isOutput=False)
    emaskp = nc.declare_dram_parameter("emask", [P, 8], F32, isOutput=False)
    GS = nc.declare_dram_parameter("GS", [SLOTS1, D], BF16, isOutput=True)
    Rout = nc.declare_dram_parameter("R", [SLOTS1, D], F32, isOutput=True)
    T = nc.dram_tensor("T", [MPAD, 2 * D], F32)

    with TileContext(nc) as tc:
        with (
            tc.tile_pool(name="const", bufs=1) as cpool,
            tc.tile_pool(name="zb", bufs=8) as zbpool,
            tc.tile_pool(name="gz", bufs=3) as gzpool,
            tc.tile_pool(name="work", bufs=6) as wpool,
            tc.tile_pool(name="psum", bufs=2, space="PSUM") as ppool,
            tc.tile_pool(name="psumx", bufs=4, space="PSUM") as pxpool,
        ):
            wx = cpool.tile([P, D], F32)
            nc.sync.dma_start(out=wx[:], in_=WXp[:])
            wa = cpool.tile([P, D], F32)
            nc.sync.dma_start(out=wa[:], in_=WAp[:])
            wz = cpool.tile([P, D], F32)
            nc.sync.dma_start(out=wz[:], in_=WZp[:])
            bcf = cpool.tile([P, MPAD], F32)
            nc.sync.dma_start(out=bcf[:], in_=bcfT[:])
            asb = cpool.tile([P, NGRP1 * (NI1 // 16)], I16)
            dcnt = cpool.tile([P, 8], F32)
            nc.sync.dma_start(out=dcnt[:], in_=dcntp[:])

            rcc = cpool.tile([P, 8], F32)
            nc.vector.reciprocal(rcc[:], dcnt[:])
            dcol = cpool.tile([P, 8], F32)
            nc.scalar.activation(dcol[:], rcc[:], ACT.Sqrt)      # dcol
            dcol3 = cpool.tile([P, 8], F32)
            nc.scalar.activation(dcol3[:], rcc[:], ACT.Sqrt, scale=1.0 / 9.0)

            # broadcaster tables: T[:, :D] = dcol*(bcf@Walpha); T[:, D:] = (dcol/3)*(bcf@WZ)
            tzb_list = []
            tzzb_list = []
            for jj in range(8):
                pz = ppool.tile([P, D], F32, space="PSUM", tag="pz")
                nc.tensor.matmul(
                    out=pz[:], lhsT=bcf[:, jj * P:(jj + 1) * P], rhs=wa[:],
                    start=True, stop=True,
                )
                tzb = zbpool.tile([P, D], F32, tag="tzb")
                nc.vector.tensor_scalar(
                    out=tzb[:], in0=pz[:], scalar1=dcol[:, jj:jj + 1],
                    scalar2=None, op0=AOP.mult,
                )
                nc.sync.dma_start(out=T[jj * P:(jj + 1) * P, 0:D], in_=tzb[:])
                pz2 = ppool.tile([P, D], F32, space="PSUM", tag="pz2")
                nc.tensor.matmul(
                    out=pz2[:], lhsT=bcf[:, jj * P:(jj + 1) * P], rhs=wz[:],
                    start=True, stop=True,
                )
                tzzb = zbpool.tile([P, D], F32, tag="tzzb")
                nc.vector.tensor_scalar(
                    out=tzzb[:], in0=pz2[:], scalar1=dcol3[:, jj:jj + 1],
                    scalar2=None, op0=AOP.mult,
                )
                nc.sync.dma_start(out=T[jj * P:(jj + 1) * P, D:2 * D], in_=tzzb[:])
                tzb_list.append(tzb)
                tzzb_list.append(tzzb)

            # T is read back by dma_gather below; order explicitly since Tile
            # does not track raw DRAM tensors.
            tc.strict_bb_all_engine_barrier()

            # bulk x load + per-window scalars overlap the first bc-gathers
            nc.sync.dma_start(out=asb[:], in_=aidx[:])
            deg = cpool.tile([P, NW1], F32)
            nc.sync.dma_start(out=deg[:], in_=degp[:])
            emask = cpool.tile([P, 8], F32)
            nc.sync.dma_start(out=emask[:], in_=emaskp[:])
            dm1 = cpool.tile([P, 8], F32)
            nc.vector.tensor_scalar_mul(dm1[:], emask[:], C1 - 1.0)
            em2 = cpool.tile([P, 8], F32)
            nc.vector.tensor_scalar_mul(em2[:], emask[:], C1)
            xsb = cpool.tile([P, SLOTS1], F32)
            nc.sync.dma_start(out=xsb[:], in_=xT[:])
            rec = cpool.tile([P, NW1], F32)
            nc.vector.reciprocal(rec[:], deg[:])
            dsb = cpool.tile([P, NW1], F32)
            nc.scalar.activation(dsb[:], rec[:], ACT.Sqrt)       # d
            dd = cpool.tile([P, NW1], F32)
            nc.vector.tensor_scalar_mul(dd[:], rec[:], INV3)     # d^2/3

            gz = None
            for j in range(NW1):
                g_grp, k = divmod(j, GRP1)
                if k == 0:
                    gz = gzpool.tile([P, GRP1 * 2 * D], F32)
                    nc.gpsimd.dma_gather(
                        out_ap=gz[:].rearrange("p (c r) -> p c r", c=GRP1),
                        in_ap=T[:, :],
                        idxs_ap=asb[:, g_grp * (NI1 // 16):(g_grp + 1) * (NI1 // 16)],
                        num_idxs=NI1, num_idxs_reg=NI1, elem_size=2 * D,
                        single_packet=False,
                    )
                zba = gz[:, k * 2 * D: k * 2 * D + D]
                zzba = gz[:, k * 2 * D + D: (k + 1) * 2 * D]

                px = pxpool.tile([P, D], F32, space="PSUM", tag="px")
                nc.tensor.matmul(
                    out=px[:], lhsT=xsb[:, j * P:(j + 1) * P], rhs=wx[:],
                    start=True, stop=True,
                )
                g_t = wpool.tile([P, D], F32, tag="g")
                nc.vector.tensor_add(out=g_t[:], in0=px[:], in1=zba)
                rin = zzba
                if j < 8:
                    f1 = wpool.tile([P, D], F32, tag="f1")
                    nc.vector.tensor_scalar(
                        out=f1[:], in0=zba, scalar1=dm1[:, j:j + 1],
                        scalar2=None, op0=AOP.mult,
                    )
                    f2 = wpool.tile([P, D], F32, tag="f2")
                    nc.vector.tensor_scalar(
                        out=f2[:], in0=tzb_list[j][:], scalar1=em2[:, j:j + 1],
                        scalar2=None, op0=AOP.mult,
                    )
                    nc.vector.tensor_add(out=g_t[:], in0=g_t[:], in1=f1[:])
                    nc.vector.tensor_add(out=g_t[:], in0=g_t[:], in1=f2[:])
                    rf1 = wpool.tile([P, D], F32, tag="rf1")
                    nc.vector.tensor_scalar(
                        out=rf1[:], in0=zzba, scalar1=dm1[:, j:j + 1],
                        scalar2=None, op0=AOP.mult,
                    )
                    rf2 = wpool.tile([P, D], F32, tag="rf2")
                    nc.vector.tensor_scalar(
                        out=rf2[:], in0=tzzb_list[j][:], scalar1=em2[:, j:j + 1],
                        scalar2=None, op0=AOP.mult,
                    )
                    rin_t = wpool.tile([P, D], F32, tag="rin")
                    nc.vector.tensor_add(out=rin_t[:], in0=zzba, in1=rf1[:])
                    nc.vector.tensor_add(out=rin_t[:], in0=rin_t[:], in1=rf2[:])
                    rin = rin_t[:]

                # Gs = d*G (bf16 out); kept on DVE so the whole per-window
                # chain px->g->gs->rt runs on one engine without cross-engine
                # semaphore hops (the window pacing is latency-bound).
                gs_t = wpool.tile([P, D], BF16, tag="gs")
                nc.vector.tensor_scalar(
                    out=gs_t[:], in0=g_t[:], scalar1=dsb[:, j:j + 1],
                    scalar2=None, op0=AOP.mult,
                )
                nc.sync.dma_start(out=GS[j * P:(j + 1) * P, :], in_=gs_t[:])
                # R = (d^2/3)*G + rin fused on DVE
                rt = wpool.tile([P, D], F32, tag="rt")
                nc.vector.scalar_tensor_tensor(
                    out=rt[:], in0=g_t[:], scalar=dd[:, j:j + 1], in1=rin,
                    op0=AOP.mult, op1=AOP.add,
                )
                nc.sync.dma_start(out=Rout[j * P:(j + 1) * P, :], in_=rt[:])

    nc.compile()
    return nc


def _build_launch2(cb):
    CAP, NCH, STREAM, CALLS_PB = _geom(cb)
    nc = bacc.Bacc()
    GSp = nc.declare_dram_parameter("GS", [N, D], BF16, isOutput=False)
    # per-bank idx streams, wrap16 per call: [P, NB * CALLS_PB * (NI2//16)]
    idxp = nc.declare_dram_parameter("idx16", [P, NB * STREAM // 16], I16,
                                     isOutput=False)
    WCH = NB * CB               # one-hot chunks per window (16)
    OHp = nc.declare_dram_parameter("OH", [NW * P, WCH * D], BF16,
                                    isOutput=False)
    degwp = nc.declare_dram_parameter("degw", [P, NW], F32, isOutput=False)
    Rwp = nc.declare_dram_parameter("Rw", [SLOTS, D], F32, isOutput=False)
    OUT = nc.declare_dram_parameter("OUT", [SLOTS, D], F32, isOutput=True)

    CHC = NI2 // P              # chunks per call (50)
    WPC = NI2 // 16             # idx columns per call (400)

    with TileContext(nc) as tc:
        with (
            tc.tile_pool(name="const", bufs=1) as cpool,
            tc.tile_pool(name="msg", bufs=3) as msgp,
            tc.tile_pool(name="oh", bufs=2) as ohp,
            tc.tile_pool(name="fin", bufs=4) as finp,
            tc.tile_pool(name="psum", bufs=4, space="PSUM") as ppool,
        ):
            idx = cpool.tile([P, NB * STREAM // 16], I16)
            nc.sync.dma_start(out=idx[:], in_=idxp[:])
            degw = cpool.tile([P, NW], F32)
            nc.sync.dma_start(out=degw[:], in_=degwp[:])
            rec = cpool.tile([P, NW], F32)
            nc.vector.reciprocal(rec[:], degw[:])
            dsc = cpool.tile([P, NW], F32)
            nc.scalar.activation(dsc[:], rec[:], ACT.Sqrt, scale=1.0 / 9.0)  # d/3

            # per-bank double-buffered message tiles; call k of bank q
            # gathers stream positions [k*NI2, (k+1)*NI2) of bank q.
            msgt = {}

            def issue_call(q, k):
                t = msgp.tile([P, CHC * D], BF16, tag=f"m{q}")
                nc.gpsimd.dma_gather(
                    out_ap=t[:].rearrange("p (c r) -> p c r", c=CHC),
                    in_ap=GSp[q * BANK:(q + 1) * BANK, :],
                    idxs_ap=idx[:, (q * CALLS_PB + k) * WPC:
                                (q * CALLS_PB + k + 1) * WPC],
                    num_idxs=NI2, num_idxs_reg=NI2, elem_size=D,
                    single_packet=False,
                )
                msgt[(q, k)] = t

            # prime: first two calls of each bank (bufs=3 per bank)
            for k0 in range(2):
                for q in range(NB):
                    issue_call(q, k0)

            issued = {q: 2 for q in range(NB)}
            for w in range(NW):
                oh = ohp.tile([P, WCH * D], BF16)
                nc.sync.dma_start(out=oh[:], in_=OHp[w * P:(w + 1) * P, :])
                ps = ppool.tile([P, D], F32, space="PSUM")
                for q in range(NB):
                    for j in range(CB):
                        c = w * CB + j                  # chunk idx in bank q
                        k, cc = divmod(c, CHC)
                        if k >= issued[q]:
                            issue_call(q, k)
                            issued[q] = k + 1
                        nc.tensor.matmul(
                            out=ps[:], lhsT=oh[:, (q * CB + j) * D:
                                               (q * CB + j + 1) * D],
                            rhs=msgt[(q, k)][:, cc * D:(cc + 1) * D],
                            start=(q == 0 and j == 0),
                            stop=(q == NB - 1 and j == CB - 1),
                        )
                rw = finp.tile([P, D], F32, tag="rw")
                nc.sync.dma_start(out=rw[:], in_=Rwp[w * P:(w + 1) * P, :])
                o2 = finp.tile([P, D], F32, tag="o2")
                nc.vector.scalar_tensor_tensor(
                    out=o2[:], in0=ps[:], scalar=dsc[:, w:w + 1], in1=rw[:],
                    op0=AOP.mult, op1=AOP.add,
                )
                nc.sync.dma_start(out=OUT[w * P:(w + 1) * P, :], in_=o2[:])

    nc.compile()
    return nc


def _get_kernels(cb):
    if "l1" not in _kernel_cache:
        _kernel_cache["l1"] = _build_launch1()
    if ("l2", cb) not in _kernel_cache:
        _kernel_cache[("l2", cb)] = _build_launch2(cb)
    return _kernel_cache["l1"], _kernel_cache[("l2", cb)]


def _pack_slots(vec, pad_value, ncols):
    """[values] -> [P, ncols] with flat index col*128+p."""
    tmp = np.full(ncols * P, pad_value, dtype=vec.dtype)
    tmp[: len(vec)] = vec
    return np.ascontiguousarray(tmp.reshape(ncols, P).T)


def _pack_windows(bdeg, CAP):
    """Assign local nodes to NW windows s.t. every (window, bank) cell has
    <= CAP edges and every window <= P nodes.  bdeg: [NC, NB] int per-node
    per-bank in-degree.  Returns (wwin, wslot) or None if infeasible."""
    order = np.argsort(-bdeg.sum(axis=1), kind="stable")
    used = np.zeros((NW, NB), dtype=np.int64)
    slots_used = np.zeros(NW, dtype=np.int64)
    wwin = np.empty(NC, dtype=np.int64)
    wslot = np.empty(NC, dtype=np.int64)
    for n in order:
        dv = bdeg[n]
        fit = ((used + dv) <= CAP).all(axis=1) & (slots_used < P)
        if not fit.any():
            return None
        cand = np.where(fit)[0]
        # least-loaded feasible window by max resulting bank load
        score = (used[cand] + dv).max(axis=1) * 1000 + slots_used[cand]
        w = cand[np.argmin(score)]
        wwin[n] = w
        wslot[n] = slots_used[w]
        used[w] += dv
        slots_used[w] += 1
    return wwin, wslot


def _prep_core(c, row_s, col_s, bounds, deg, cb):
    """Host integer work for core c: window packing + per-bank gather
    streams in fixed cb-chunk layout."""
    CAP, NCH, STREAM, CALLS_PB = _geom(cb)
    lo, hi = bounds[c * NC], bounds[(c + 1) * NC]
    edest = row_s[lo:hi] - c * NC          # local dest node of each edge
    ecol = col_s[lo:hi]                    # global source node
    q_e = ecol // BANK

    # per-node per-bank degree
    bdeg = np.zeros((NC, NB), dtype=np.int64)
    np.add.at(bdeg, (edest, q_e), 1)

    res = _pack_windows(bdeg, CAP)
    if res is None:
        return None
    wwin, wslot = res

    w_e = wwin[edest]
    s_e = wslot[edest]
    rel_e = (ecol - q_e * BANK).astype(np.int16)

    # per-bank streams: position = w*CAP + rank within (w, q)
    idx_stream = np.zeros((NB, STREAM), dtype=np.int16)
    # one-hot tiles: row w*P + p, col (q*CB + j)*D + slot
    WCH = NB * CB
    OH = np.zeros((NW * P, WCH * D), dtype=BF16NP)
    for q in range(NB):
        m = q_e == q
        wq = w_e[m]
        order = np.argsort(wq, kind="stable")
        wq_s = wq[order]
        rel_s = rel_e[m][order]
        s_s = s_e[m][order]
        # rank within each window run
        starts = np.searchsorted(wq_s, np.arange(NW))
        counts = np.searchsorted(wq_s, np.arange(NW), side="right") - starts
        rank = np.arange(len(wq_s)) - np.repeat(starts, counts)
        pos = wq_s * CAP + rank
        idx_stream[q, pos] = rel_s
        r_in = pos % CAP
        OH[wq_s * P + (r_in % P), (q * CB + r_in // P) * D + s_s] = 1.0

    idx16 = np.concatenate(
        [_wrap16(idx_stream[q, k * NI2:(k + 1) * NI2], NI2)
         for q in range(NB) for k in range(CALLS_PB)],
        axis=1,
    )

    perm = np.full(SLOTS, -1, dtype=np.int64)
    perm[wwin * P + wslot] = np.arange(c * NC, (c + 1) * NC)

    degw_flat = np.ones(SLOTS, dtype=np.float32)
    valid = perm >= 0
    degw_flat[valid] = deg[perm[valid]].astype(np.float32)
    degw = np.ascontiguousarray(degw_flat.reshape(NW, P).T)

    return {"idx16": idx16, "OH": OH, "perm": perm, "degw": degw}


def kernel(x, edge_index, bc_feature, bc_assignment, WX, WZ, Walpha):
    x = np.asarray(x, dtype=np.float32)
    edge_index = np.asarray(edge_index)
    bc_feature = np.asarray(bc_feature, dtype=np.float32)
    bc_assignment = np.asarray(bc_assignment)
    WX = np.asarray(WX, dtype=np.float32)
    WZ = np.asarray(WZ, dtype=np.float32)
    Walpha = np.asarray(Walpha, dtype=np.float32)

    row = edge_index[0].astype(np.int64)
    col = edge_index[1].astype(np.int64)
    assign = bc_assignment.astype(np.int64)

    deg = np.bincount(col, minlength=N).astype(np.int64) + 1
    cnt = np.bincount(assign, minlength=M).astype(np.int64) + 1

    order = np.argsort(row, kind="stable")
    row_s = row[order]
    col_s = col[order]
    bounds = np.searchsorted(row_s, np.arange(N + 1))

    cb = CB
    cores = [_prep_core(c, row_s, col_s, bounds, deg, cb) for c in range(NCORES)]
    if any(ci is None for ci in cores):
        cb = CB_FALLBACK
        cores = [_prep_core(c, row_s, col_s, bounds, deg, cb)
                 for c in range(NCORES)]
        if any(ci is None for ci in cores):
            raise RuntimeError("window packing infeasible")

    nc1, nc2 = _get_kernels(cb)

    # ---------------- launch 1 ----------------
    NI1 = GRP1 * P
    bcfT = np.zeros((P, MPAD), dtype=np.float32)
    bcfT[:, :M] = bc_feature.T
    dcnt_sb = _pack_slots(cnt.astype(np.float32), np.float32(1.0), 8)
    in_maps1 = []
    for c in range(NCORES):
        xTc = np.zeros((P, SLOTS1), dtype=np.float32)
        xTc[:, :NC] = x[c * NC:(c + 1) * NC].T
        a_pad = np.zeros(SLOTS1, dtype=np.int16)
        a_pad[:NC] = assign[c * NC:(c + 1) * NC].astype(np.int16)
        aidx16 = np.concatenate(
            [_wrap16(a_pad[g * NI1:(g + 1) * NI1], NI1) for g in range(NGRP1)],
            axis=1,
        )
        deg_sb = _pack_slots(deg[c * NC:(c + 1) * NC].astype(np.float32),
                             np.float32(1.0), NW1)
        em = np.zeros(MPAD, dtype=np.float32)
        gids = c * NC + np.arange(MPAD)
        em[gids < M] = 1.0
        emask_sb = np.ascontiguousarray(em.reshape(8, P).T)
        in_maps1.append({
            "xT": xTc,
            "WX": WX, "WA": Walpha, "WZ": WZ,
            "bcfT": bcfT,
            "aidx16": aidx16,
            "deg": deg_sb,
            "dcnt": dcnt_sb,
            "emask": emask_sb,
        })

    res1 = run_bass_kernel_spmd(nc1, in_maps1, core_ids=CORE_IDS)
    LAST_RESULTS.clear()
    LAST_RESULTS.append(res1)

    GS = np.concatenate(
        [np.asarray(res1.results[c]["GS"])[:NC] for c in range(NCORES)], axis=0
    )
    GS = np.ascontiguousarray(GS.astype(BF16NP))

    # ---------------- launch 2 ----------------
    in_maps2 = []
    for c in range(NCORES):
        ci = cores[c]
        R_c = np.asarray(res1.results[c]["R"])[:NC]
        Rw = np.zeros((SLOTS, D), dtype=np.float32)
        valid = ci["perm"] >= 0
        Rw[valid] = R_c[ci["perm"][valid] - c * NC]
        in_maps2.append({
            "GS": GS,
            "idx16": ci["idx16"],
            "OH": ci["OH"],
            "degw": ci["degw"],
            "Rw": Rw,
        })

    res2 = run_bass_kernel_spmd(nc2, in_maps2, core_ids=CORE_IDS)
    LAST_RESULTS.append(res2)

    out = np.empty((N, D), dtype=np.float32)
    for c in range(NCORES):
        ci = cores[c]
        valid = ci["perm"] >= 0
        out[ci["perm"][valid]] = np.asarray(res2.results[c]["OUT"])[valid]
    return out
